# revision 1
# baseline (speedup 1.0000x reference)
"""B2Bsqrt-TANDEM LSTM kernel for Trainium2 (8 NeuronCores, data-parallel over batch).

Reference computation (per core, batch shard BL=64):
  xz = einsum('btf,gfh->tgbh', x, W) + b
  per step: z = xz_t + h @ U ; i,f,o = sigmoid(z_ifo); ct = b2bsqrt(z_c)
            c = f*c + i*ct ; h = o * b2bsqrt(c)
  LayerNorm(h) then Linear -> logits (B, T, 10)

Layout: per gate g, the two 512-wide halves of z_g are computed as a
column-tiled matmul pair — half A with tile_position (0,0) into PSUM
partitions 0:64, half B with (0,64) into partitions 64:128 — so the two
M=64 matmuls run concurrently in disjoint PE column groups. All gate /
cell / h tensors then live in this folded (128, 512) layout (row p<64 =
batch p cols 0:512, row p>=64 = batch p-64 cols 512:1024), which halves
every ACT/DVE pass. Inputs bf16, PSUM fp32. The xz part of the first
PREFILL gates of step t+1 is issued during step t so the PE never idles
across the recurrence boundary (keeps the HAM clock warm). h is
transposed back to h-major via PE transposes. LN+FC fused:
logits = rsig*(h@gw - mu*u) + (ln_b@fc_w + fc_b); mu/sumsq come from
tensor_tensor_reduce accumulators folded with a small SBUF-to-SBUF DMA.
ACT ops are chained in emission order so the activation table switches
exactly twice per step (sigmoid set <-> sqrt set).
"""

import os
import sys

sys.path.insert(0, "/opt/trn_rl_repo")

import numpy as np
import ml_dtypes

import concourse.bass as bass
import concourse.mybir as mybir
import concourse.tile as tile
from concourse import bacc
from concourse.bass_utils import run_bass_kernel_spmd
from concourse.masks import make_identity
from concourse.tile_rust import add_dep_helper

AF = mybir.ActivationFunctionType
OP = mybir.AluOpType
BF16 = mybir.dt.bfloat16
F32 = mybir.dt.float32

N_CORES = 8
B_FULL = 512
BL = B_FULL // N_CORES  # 64 batch rows per core
T_FULL = 100
H = 1024
G4 = 4 * H  # 4096
C = 10
KC = H // 128  # 8 contraction chunks
LN_EPS = 1e-5
PREFILL = 3  # xz gate-pairs of step t+1 issued during step t

GATE_ORDER = [0, 1, 3, 2]  # i, f, ct, o


def emit(ctx, nc, tc, T, with_bias):
    sing = ctx.enter_context(tc.tile_pool(name="sing", bufs=1))
    xt_pool = ctx.enter_context(tc.tile_pool(name="xt", bufs=3))
    ht_pool = ctx.enter_context(tc.tile_pool(name="ht", bufs=2))
    gp = ctx.enter_context(tc.tile_pool(name="gp", bufs=1))
    sp = ctx.enter_context(tc.tile_pool(name="sp", bufs=2))
    zp = ctx.enter_context(tc.tile_pool(name="zp", bufs=4, space="PSUM"))
    tp = ctx.enter_context(tc.tile_pool(name="tp", bufs=2, space="PSUM"))
    fp = ctx.enter_context(tc.tile_pool(name="fp", bufs=1, space="PSUM"))
    lp = ctx.enter_context(tc.tile_pool(name="lp", bufs=1, space="PSUM"))

    dW = nc.dram_tensor("Wn", [KC, 128, G4], BF16, kind="ExternalInput")
    dU = nc.dram_tensor("Un", [KC, 128, G4], BF16, kind="ExternalInput")
    dX = nc.dram_tensor("xT", [T, KC, 128, BL], BF16, kind="ExternalInput")
    dGW = nc.dram_tensor("gw", [KC, 128, C], BF16, kind="ExternalInput")
    dUB = nc.dram_tensor("ub", [BL, C], F32, kind="ExternalInput")
    dVB = nc.dram_tensor("vbb", [BL, C], F32, kind="ExternalInput")
    if with_bias:
        dBB = nc.dram_tensor("bb", [128, G4], BF16, kind="ExternalInput")
    dOUT = nc.dram_tensor("out", [BL, T * C], F32, kind="ExternalOutput")

    # --- resident weights / constants ---
    W_sb = sing.tile([128, KC, G4], BF16)
    nc.sync.dma_start(W_sb[:], dW.rearrange("k p n -> p k n"))
    U_sb = sing.tile([128, KC, G4], BF16)
    nc.sync.dma_start(U_sb[:], dU.rearrange("k p n -> p k n"))
    gw_sb = sing.tile([128, KC, C], BF16)
    nc.sync.dma_start(gw_sb[:], dGW.rearrange("k p c -> p k c"))
    ub_sb = sing.tile([BL, C], F32)
    nc.sync.dma_start(ub_sb[:], dUB[:])
    vb_sb = sing.tile([BL, C], F32)
    nc.sync.dma_start(vb_sb[:], dVB[:])
    if with_bias:
        bb_sb = sing.tile([128, G4], BF16)
        nc.sync.dma_start(bb_sb[:], dBB[:])
        ones_col = sing.tile([128, BL], BF16)
        nc.vector.memset(ones_col[:], 0.0)
        nc.vector.memset(ones_col[0:1, :], 1.0)

    id64 = sing.tile([128, BL], BF16)  # identity in both partition halves
    make_identity(nc, id64[0:BL, :])
    make_identity(nc, id64[BL:128, :])
    id10 = sing.tile([C, C], F32)
    make_identity(nc, id10[:])

    eps_sb = sing.tile([BL, 1], F32)
    nc.vector.memset(eps_sb[:], LN_EPS)

    c_st = sing.tile([128, 512], BF16)
    nc.vector.memset(c_st[:], 0.0)
    hT0 = sing.tile([128, KC, BL], BF16)
    nc.vector.memset(hT0[:], 0.0)
    logit_acc = sing.tile([BL, T * C], F32)

    hT_prev = hT0
    act_chain = [None]

    def act(*args, **kwargs):
        inst = nc.scalar.activation(*args, **kwargs)
        if act_chain[0] is not None:
            add_dep_helper(inst.ins, act_chain[0].ins, False,
                           "act table order")
        act_chain[0] = inst
        return inst

    xts = {}

    def get_xt(t):
        if t not in xts:
            xt = xt_pool.tile([128, KC, BL], BF16, tag="xt")
            nc.sync.dma_start(xt[:], dX[t].rearrange("k p b -> p k b"))
            xts[t] = xt
        return xts[t]

    def mm_pair(z_ps, lhsT_k, rhs, g, k, start, stop=False):
        """One col-tiled pair: half A -> partitions 0:64, half B -> 64:128."""
        nsA = slice(g * 1024, g * 1024 + 512)
        nsB = slice(g * 1024 + 512, (g + 1) * 1024)
        ia = nc.tensor.matmul(z_ps[0:BL, :], lhsT_k, rhs[:, k, nsA],
                              start=start, stop=stop, tile_position=(0, 0))
        ib = nc.tensor.matmul(z_ps[BL:128, :], lhsT_k, rhs[:, k, nsB],
                              start=start, stop=stop, tile_position=(0, 64),
                              skip_group_check=True)
        return ia, ib

    def emit_xz(t, g):
        """xz part of gate g of step t into a fresh folded PSUM tile."""
        z_ps = zp.tile([128, 512], F32, tag="z")
        xt = get_xt(t)
        for k in range(KC):
            mm_pair(z_ps, xt[:, k, :], W_sb, g, k, start=(k == 0))
        if with_bias:
            nsA = slice(g * 1024, g * 1024 + 512)
            nsB = slice(g * 1024 + 512, (g + 1) * 1024)
            nc.tensor.matmul(z_ps[0:BL, :], ones_col[:], bb_sb[:, nsA],
                             start=False, stop=False, tile_position=(0, 0))
            nc.tensor.matmul(z_ps[BL:128, :], ones_col[:], bb_sb[:, nsB],
                             start=False, stop=False, tile_position=(0, 64),
                             skip_group_check=True)
        return z_ps

    pending = {}

    for t in range(T):
        sig_i = gp.tile([128, 512], BF16, tag="sig_i")
        sig_f = gp.tile([128, 512], BF16, tag="sig_f")
        sig_o = gp.tile([128, 512], BF16, tag="sig_o")
        a3 = gp.tile([128, 512], BF16, tag="a3")
        sg3 = gp.tile([128, 512], BF16, tag="sg3")

        # --- z gates: accumulate h@U on top of prefilled xz, then gates ---
        for idx, g in enumerate(GATE_ORDER):
            z_ps = pending.pop(g, None)
            if z_ps is None:
                z_ps = emit_xz(t, g)
            for k in range(KC):
                mm_pair(z_ps, hT_prev[:, k, :], U_sb, g, k, start=False,
                        stop=(k == KC - 1))
            # nonlinearity straight from PSUM (sigmoid-table phase ops)
            if g == 0:
                act(sig_i[:], z_ps[:], AF.Sigmoid)
            elif g == 1:
                act(sig_f[:], z_ps[:], AF.Sigmoid)
            elif g == 2:
                act(sig_o[:], z_ps[:], AF.Sigmoid)
            else:  # c~ pre-parts: |z|, sign(z)
                act(a3[:], z_ps[:], AF.Abs)
                act(sg3[:], z_ps[:], AF.Sign)
            # keep the PE busy over the step boundary: prefill xz of t+1
            # (emitted after the last gate so it lands in the c->h window)
            if t + 1 < T and idx == len(GATE_ORDER) - 1:
                for pg in GATE_ORDER[:PREFILL]:
                    pending[pg] = emit_xz(t + 1, pg)

        # --- sqrt-table phase ---
        s3 = gp.tile([128, 512], F32, tag="s3")
        act(s3[:], a3[:], AF.Sqrt, bias=1.0)  # sqrt(1+|z|)
        nc.vector.tensor_scalar(s3[:], s3[:], 1.0, None, OP.subtract)
        ctld = gp.tile([128, 512], BF16, tag="ctld")
        nc.vector.tensor_tensor(ctld[:], s3[:], sg3[:], OP.mult)

        # c = f*c + i*ct
        tmp1 = gp.tile([128, 512], BF16, tag="tmp1")
        nc.vector.tensor_tensor(tmp1[:], sig_f[:], c_st[:], OP.mult)
        tmp2 = gp.tile([128, 512], BF16, tag="tmp2")
        nc.vector.tensor_tensor(tmp2[:], sig_i[:], ctld[:], OP.mult)
        nc.vector.tensor_tensor(c_st[:], tmp1[:], tmp2[:], OP.add)

        # h = o * sign(c) * (sqrt(1+|c|)-1), with sum(h) accumulated
        ac = gp.tile([128, 512], BF16, tag="ac")
        act(ac[:], c_st[:], AF.Abs)
        sc = gp.tile([128, 512], F32, tag="sc")
        act(sc[:], ac[:], AF.Sqrt, bias=1.0)
        sgc = gp.tile([128, 512], BF16, tag="sgc")
        act(sgc[:], c_st[:], AF.Sign)
        nc.vector.tensor_scalar(sc[:], sc[:], 1.0, None, OP.subtract)
        hsg = gp.tile([128, 512], BF16, tag="hsg")
        nc.vector.tensor_tensor(hsg[:], sc[:], sgc[:], OP.mult)
        st2 = sp.tile([128, 2], F32, tag="st2")
        h_bf = gp.tile([128, 512], BF16, tag="h_bf")
        nc.vector.tensor_tensor(h_bf[:], hsg[:], sig_o[:], OP.mult)
        scr = gp.tile([128, 512], BF16, tag="scr")
        act(scr[:], h_bf[:], AF.Copy, accum_out=st2[:, 0:1])
        act(scr[:], h_bf[:], AF.Square, accum_out=st2[:, 1:2])

        # transpose h -> hT (PE transpose per 128-col block of unfolded h)
        hT = ht_pool.tile([128, KC, BL], BF16, tag="hT")
        for k in range(KC):
            if k < 4:
                src = h_bf[0:BL, k * 128:(k + 1) * 128]
                ident = id64[0:BL, :]
            else:
                src = h_bf[BL:128, (k - 4) * 128:(k - 3) * 128]
                ident = id64[BL:128, :]
            t_ps = tp.tile([128, BL], BF16, tag="tps")
            nc.tensor.transpose(t_ps[:], src, ident)
            nc.vector.tensor_copy(hT[:, k, :], t_ps[:])

        # FC: raw.T = gw.T @ hT  (10, 64)
        f_ps = fp.tile([C, BL], F32, tag="fps")
        for k in range(KC):
            nc.tensor.matmul(f_ps[:], gw_sb[:, k, :], hT[:, k, :],
                             start=(k == 0), stop=(k == KC - 1))
        fc_sb = sp.tile([C, BL], F32, tag="fc_sb")
        nc.vector.tensor_copy(fc_sb[:], f_ps[:])
        l_ps = lp.tile([BL, C], F32, tag="lps")
        nc.tensor.transpose(l_ps[:], fc_sb[:], id10[:])

        # fold stats: per-batch sums = rows 0:64 + rows 64:128
        st_lo = sp.tile([BL, 2], F32, tag="st_lo")
        nc.sync.dma_start(st_lo[:], st2[BL:128, :])
        sums = sp.tile([BL, 2], F32, tag="sums")
        nc.vector.tensor_tensor(sums[:], st2[0:BL, :], st_lo[:], OP.add)

        # mu = sumh/H ; var = sumsq/H - mu^2 ; rsig = 1/sqrt(var+eps)
        mu = sp.tile([BL, 1], F32, tag="mu")
        nc.vector.tensor_scalar(mu[:], sums[:, 0:1], 1.0 / H, None, OP.mult)
        musq = sp.tile([BL, 1], F32, tag="musq")
        nc.vector.tensor_tensor(musq[:], mu[:], mu[:], OP.mult)
        var = sp.tile([BL, 1], F32, tag="var")
        nc.vector.tensor_scalar(var[:], sums[:, 1:2], 1.0 / H, None, OP.mult)
        nc.vector.tensor_tensor(var[:], var[:], musq[:], OP.subtract)
        sd = sp.tile([BL, 1], F32, tag="sd")
        act(sd[:], var[:], AF.Sqrt, bias=eps_sb[:])
        rsig = sp.tile([BL, 1], F32, tag="rsig")
        nc.vector.reciprocal(rsig[:], sd[:])

        # logits = rsig*(raw - mu*u) + vbb
        t3 = sp.tile([BL, C], F32, tag="t3")
        nc.vector.tensor_scalar_mul(t3[:], ub_sb[:], mu[:])
        t4 = sp.tile([BL, C], F32, tag="t4")
        nc.vector.tensor_tensor(t4[:], l_ps[:], t3[:], OP.subtract)
        nc.vector.tensor_scalar_mul(t4[:], t4[:], rsig[:])
        nc.vector.tensor_tensor(logit_acc[:, t * C:(t + 1) * C], t4[:], vb_sb[:],
                                OP.add)

        hT_prev = hT
        xts.pop(t, None)

    nc.sync.dma_start(dOUT[:], logit_acc[:])


def build(T=T_FULL, with_bias=False):
    from contextlib import ExitStack

    nc = bacc.Bacc("TRN2", target_bir_lowering=False)
    with tile.TileContext(nc) as tc:
        with ExitStack() as ctx:
            emit(ctx, nc, tc, T, with_bias)
    nc.compile()
    return nc


def kernel(x, W, U, b, ln_g, ln_b, fc_w, fc_b, _T=None, _trace=False):
    x = np.asarray(x, dtype=np.float32)
    W = np.asarray(W, dtype=np.float32)
    U = np.asarray(U, dtype=np.float32)
    b = np.asarray(b, dtype=np.float32)
    ln_g = np.asarray(ln_g, dtype=np.float32)
    ln_b = np.asarray(ln_b, dtype=np.float32)
    fc_w = np.asarray(fc_w, dtype=np.float32)
    fc_b = np.asarray(fc_b, dtype=np.float32)

    T = _T or x.shape[1]
    with_bias = bool(np.any(b))

    W_all = np.concatenate([W[g] for g in range(4)], axis=1)  # (H, 4H)
    U_all = np.concatenate([U[g] for g in range(4)], axis=1)
    Wn = np.ascontiguousarray(
        W_all.reshape(KC, 128, G4)).astype(ml_dtypes.bfloat16)
    Un = np.ascontiguousarray(
        U_all.reshape(KC, 128, G4)).astype(ml_dtypes.bfloat16)
    gw = (ln_g[:, None] * fc_w).reshape(KC, 128, C).astype(ml_dtypes.bfloat16)
    u_vec = (ln_g @ fc_w).astype(np.float32)  # (C,)
    vb = (ln_b @ fc_w + fc_b).astype(np.float32)
    ub_b = np.broadcast_to(u_vec, (BL, C)).copy()
    vb_b = np.broadcast_to(vb, (BL, C)).copy()

    common = {"Wn": Wn, "Un": Un, "gw": gw, "ub": ub_b, "vbb": vb_b}
    if with_bias:
        b_all = np.concatenate([b[g] for g in range(4)])  # (4H,)
        bb = np.zeros((128, G4), dtype=ml_dtypes.bfloat16)
        bb[0, :] = b_all.astype(ml_dtypes.bfloat16)
        common["bb"] = bb

    in_maps = []
    for ci in range(N_CORES):
        xc = x[ci * BL:(ci + 1) * BL, :T]           # (BL, T, H)
        xT = xc.transpose(1, 2, 0)                   # (T, H, BL)
        xT = np.ascontiguousarray(xT).reshape(T, KC, 128, BL)
        in_maps.append({"xT": xT.astype(ml_dtypes.bfloat16), **common})

    nc = build(T, with_bias)
    res = run_bass_kernel_spmd(nc, in_maps, core_ids=list(range(N_CORES)),
                               trace=_trace)
    out = np.concatenate(
        [r["out"].reshape(BL, T, C) for r in res.results], axis=0)
    if _trace:
        kernel.last_exec_time_ns = res.exec_time_ns
    return out



# revision 6
# speedup vs baseline: 1.0736x; 1.0736x over previous
"""B2Bsqrt-TANDEM LSTM kernel for Trainium2 (8 NeuronCores, data-parallel over batch).

Reference computation (per core, batch shard BL=64):
  xz = einsum('btf,gfh->tgbh', x, W) + b
  per step: z = xz_t + h @ U ; i,f,o = sigmoid(z_ifo); ct = b2bsqrt(z_c)
            c = f*c + i*ct ; h = o * b2bsqrt(c)
  LayerNorm(h) then Linear -> logits (B, T, 10)

Layout: per gate g, the two 512-wide halves of z_g are computed as a
column-tiled matmul pair — half A with tile_position (0,0) into PSUM
partitions 0:64, half B with (0,64) into partitions 64:128 — so the two
M=64 matmuls stream concurrently in disjoint PE column groups. All gate /
cell / h tensors live in this folded (128, 512) layout (row p<64 = batch p
cols 0:512, row p>=64 = batch p-64 cols 512:1024).

Schedule: gates are processed in order [i, f, o, ct] so the sigmoid->sqrt
activation-table switch lands inside ct's recurrent matmul window instead
of on the serial chain. After ct's h@U, the xz projections of ALL FOUR
gates of step t+1 are queued on the PE — ~13.6us of independent work that
covers the entire serial ACT/DVE chain (sqrt(z3) -> c -> sqrt(c) -> h), so
the tensor engine never idles and stays at the 2.4 GHz pstate. abs/sign
run on the DVE (abs_max / is_ge tricks); h is computed as q*sqrt(1+|c|)-q
with q = sig_o*sign(c) prepared while the sqrt streams. h is transposed
back to H-major via four 128x128 PE transposes (each yields two k-chunks
of the folded layout). LN+FC fused: the FC weight matrix gets an extra
ones column so sum(h) falls out of the same matmul; only sum(h^2) uses an
ACT accumulator. logits = rsig*(h@gw - mu*u) + (ln_b@fc_w + fc_b).
ACT ops are chained in emission order so the activation table switches
exactly twice per step (sigmoid set <-> sqrt set).
"""

import os
import sys

sys.path.insert(0, "/opt/trn_rl_repo")

import numpy as np
import ml_dtypes

import concourse.bass as bass
import concourse.mybir as mybir
import concourse.tile as tile
from concourse import bacc
from concourse.bass_utils import run_bass_kernel_spmd
from concourse.masks import make_identity
from concourse.tile_rust import add_dep_helper

AF = mybir.ActivationFunctionType
OP = mybir.AluOpType
BF16 = mybir.dt.bfloat16
F32 = mybir.dt.float32

N_CORES = 8
B_FULL = 512
BL = B_FULL // N_CORES  # 64 batch rows per core
T_FULL = 100
H = 1024
G4 = 4 * H  # 4096
C = 10
C2 = 16  # FC output width incl. ones column (10 logits + sum(h) + pad)
KC = H // 128  # 8 contraction chunks
LN_EPS = 1e-5

GATE_ORDER = [0, 1, 2, 3]  # i, f, o, c~  (c~ last: table switch hides in its h@U)


def emit(ctx, nc, tc, T, with_bias):
    sing = ctx.enter_context(tc.tile_pool(name="sing", bufs=1))
    xt_pool = ctx.enter_context(tc.tile_pool(name="xt", bufs=3))
    ht_pool = ctx.enter_context(tc.tile_pool(name="ht", bufs=2))
    gp = ctx.enter_context(tc.tile_pool(name="gp", bufs=1))
    sp = ctx.enter_context(tc.tile_pool(name="sp", bufs=2))
    zp = ctx.enter_context(tc.tile_pool(name="zp", bufs=4, space="PSUM"))
    tp = ctx.enter_context(tc.tile_pool(name="tp", bufs=2, space="PSUM"))
    fp = ctx.enter_context(tc.tile_pool(name="fp", bufs=1, space="PSUM"))
    lp = ctx.enter_context(tc.tile_pool(name="lp", bufs=1, space="PSUM"))

    dW = nc.dram_tensor("Wn", [KC, 128, G4], BF16, kind="ExternalInput")
    dU = nc.dram_tensor("Un", [KC, 128, G4], BF16, kind="ExternalInput")
    dX = nc.dram_tensor("xT", [T, KC, 128, BL], BF16, kind="ExternalInput")
    dGW = nc.dram_tensor("gw", [KC, 128, C2], BF16, kind="ExternalInput")
    dUB = nc.dram_tensor("ub", [BL, C], F32, kind="ExternalInput")
    dVB = nc.dram_tensor("vbb", [BL, C], F32, kind="ExternalInput")
    if with_bias:
        dBB = nc.dram_tensor("bb", [128, G4], BF16, kind="ExternalInput")
    dOUT = nc.dram_tensor("out", [BL, T * C], F32, kind="ExternalOutput")

    # --- resident weights / constants ---
    W_sb = sing.tile([128, KC, G4], BF16)
    nc.sync.dma_start(W_sb[:], dW.rearrange("k p n -> p k n"))
    U_sb = sing.tile([128, KC, G4], BF16)
    nc.sync.dma_start(U_sb[:], dU.rearrange("k p n -> p k n"))
    gw_sb = sing.tile([128, KC, C2], BF16)
    nc.sync.dma_start(gw_sb[:], dGW.rearrange("k p c -> p k c"))
    ub_sb = sing.tile([BL, C], F32)
    nc.sync.dma_start(ub_sb[:], dUB[:])
    vb_sb = sing.tile([BL, C], F32)
    nc.sync.dma_start(vb_sb[:], dVB[:])
    if with_bias:
        bb_sb = sing.tile([128, G4], BF16)
        nc.sync.dma_start(bb_sb[:], dBB[:])
        ones_col = sing.tile([128, BL], BF16)
        nc.vector.memset(ones_col[:], 0.0)
        nc.vector.memset(ones_col[0:1, :], 1.0)

    id128 = sing.tile([128, 128], BF16)
    make_identity(nc, id128[:])
    id16 = sing.tile([C2, C2], F32)
    make_identity(nc, id16[:])

    eps_sb = sing.tile([BL, 1], F32)
    nc.vector.memset(eps_sb[:], LN_EPS)

    c_st = sing.tile([128, 512], BF16)
    nc.vector.memset(c_st[:], 0.0)
    hT0 = sing.tile([128, 4, 128], BF16)
    nc.vector.memset(hT0[:], 0.0)
    logit_acc = sing.tile([BL, T * C], F32)

    hT_prev = hT0
    act_chain = [None]

    def act(*args, **kwargs):
        inst = nc.scalar.activation(*args, **kwargs)
        if act_chain[0] is not None:
            add_dep_helper(inst.ins, act_chain[0].ins, False,
                           "act table order")
        act_chain[0] = inst
        return inst

    xts = {}

    def get_xt(t):
        if t not in xts:
            xt = xt_pool.tile([128, KC, BL], BF16, tag="xt")
            nc.sync.dma_start(xt[:], dX[t].rearrange("k p b -> p k b"))
            xts[t] = xt
        return xts[t]

    def hT_sl(hT, k):
        return hT[:, k, 0:BL] if k < 4 else hT[:, k - 4, BL:128]

    def mm_pair(z_ps, lhsT_k, rhs, g, k, start, stop=False):
        """One col-tiled pair: half A -> partitions 0:64, half B -> 64:128."""
        nsA = slice(g * 1024, g * 1024 + 512)
        nsB = slice(g * 1024 + 512, (g + 1) * 1024)
        nc.tensor.matmul(z_ps[0:BL, :], lhsT_k, rhs[:, k, nsA],
                         start=start, stop=stop, tile_position=(0, 0))
        nc.tensor.matmul(z_ps[BL:128, :], lhsT_k, rhs[:, k, nsB],
                         start=start, stop=stop, tile_position=(0, 64),
                         skip_group_check=True)

    def emit_xz(t, g):
        """xz part of gate g of step t into a fresh folded PSUM tile."""
        z_ps = zp.tile([128, 512], F32, tag="z")
        xt = get_xt(t)
        for k in range(KC):
            mm_pair(z_ps, xt[:, k, :], W_sb, g, k, start=(k == 0))
        if with_bias:
            nsA = slice(g * 1024, g * 1024 + 512)
            nsB = slice(g * 1024 + 512, (g + 1) * 1024)
            nc.tensor.matmul(z_ps[0:BL, :], ones_col[:], bb_sb[:, nsA],
                             start=False, stop=False, tile_position=(0, 0))
            nc.tensor.matmul(z_ps[BL:128, :], ones_col[:], bb_sb[:, nsB],
                             start=False, stop=False, tile_position=(0, 64),
                             skip_group_check=True)
        return z_ps

    pending = {}
    for pg in GATE_ORDER:
        pending[pg] = emit_xz(0, pg)

    for t in range(T):
        sig_i = gp.tile([128, 512], BF16, tag="sig_i")
        sig_f = gp.tile([128, 512], BF16, tag="sig_f")
        sig_o = gp.tile([128, 512], BF16, tag="sig_o")
        tmp1 = gp.tile([128, 512], BF16, tag="tmp1")

        # --- z gates: accumulate h@U on top of prefilled xz; sigmoids ---
        z3_ps = None
        for g in GATE_ORDER:
            z_ps = pending.pop(g)
            for k in range(KC):
                mm_pair(z_ps, hT_sl(hT_prev, k), U_sb, g, k, start=False,
                        stop=(k == KC - 1))
            if g == 0:
                act(sig_i[:], z_ps[:], AF.Sigmoid)
            elif g == 1:
                act(sig_f[:], z_ps[:], AF.Sigmoid)
                nc.vector.tensor_tensor(tmp1[:], sig_f[:], c_st[:], OP.mult)
            elif g == 2:
                act(sig_o[:], z_ps[:], AF.Sigmoid)
            else:
                z3_ps = z_ps

        # z3 readers on DVE (emitted before the prefill that reuses banks).
        # sgn3h = sign(z3)/2 via one dual-op; |z3|/2 = z3*sgn3h.
        sgn3h = gp.tile([128, 512], BF16, tag="sgn3h")
        nc.vector.tensor_scalar(sgn3h[:], z3_ps[:], 0.0, 0.5, OP.is_ge,
                                OP.subtract)
        a3h = gp.tile([128, 512], BF16, tag="a3h")
        nc.vector.tensor_tensor(a3h[:], z3_ps[:], sgn3h[:], OP.mult)

        # keep the PE busy across the serial chain: queue xz of t+1 now
        if t + 1 < T:
            for pg in GATE_ORDER:
                pending[pg] = emit_xz(t + 1, pg)

        # --- c~ = sign(z3)*(sqrt(1+|z3|)-1) ; c = f*c + i*c~ ---
        rs = gp.tile([128, 512], BF16, tag="rs")
        nc.vector.tensor_tensor(rs[:], sig_i[:], sgn3h[:], OP.mult)
        s3 = gp.tile([128, 512], F32, tag="s3")
        act(s3[:], a3h[:], AF.Sqrt, bias=1.0, scale=2.0)  # sqrt(1+|z3|)
        s3m2 = gp.tile([128, 512], BF16, tag="s3m2")
        nc.vector.tensor_scalar(s3m2[:], s3[:], 1.0, 2.0, OP.subtract,
                                OP.mult)  # 2*(s3-1)
        tmp2 = gp.tile([128, 512], BF16, tag="tmp2")
        nc.vector.tensor_tensor(tmp2[:], rs[:], s3m2[:], OP.mult)
        nc.vector.tensor_tensor(c_st[:], tmp1[:], tmp2[:], OP.add)

        # --- h = q2*2*(sqrt(1+|c|)-1), q2 = sig_o*sign(c)/2 ---
        sgnCh = gp.tile([128, 512], BF16, tag="sgnCh")
        nc.vector.tensor_scalar(sgnCh[:], c_st[:], 0.0, 0.5, OP.is_ge,
                                OP.subtract)
        aCh = gp.tile([128, 512], BF16, tag="aCh")
        nc.vector.tensor_tensor(aCh[:], c_st[:], sgnCh[:], OP.mult)
        sC = gp.tile([128, 512], F32, tag="sC")
        act(sC[:], aCh[:], AF.Sqrt, bias=1.0, scale=2.0)  # sqrt(1+|c|)
        q2 = gp.tile([128, 512], BF16, tag="q2")
        nc.vector.tensor_tensor(q2[:], sig_o[:], sgnCh[:], OP.mult)
        sCm2 = gp.tile([128, 512], BF16, tag="sCm2")
        nc.vector.tensor_scalar(sCm2[:], sC[:], 1.0, 2.0, OP.subtract,
                                OP.mult)  # 2*(sC-1), subtract done in fp32
        h_bf = gp.tile([128, 512], BF16, tag="h_bf")
        nc.vector.tensor_tensor(h_bf[:], q2[:], sCm2[:], OP.mult)

        # sum(h^2) via ACT accumulator (sum(h) rides the FC ones column)
        st2 = sp.tile([128, 1], F32, tag="st2")
        scr = gp.tile([128, 512], BF16, tag="scr")
        act(scr[:], h_bf[:], AF.Square, accum_out=st2[:, 0:1])

        # transpose h -> hT: 4 x (128,128) blocks, each covers 2 k-chunks
        hT = ht_pool.tile([128, 4, 128], BF16, tag="hT")
        for j in range(4):
            t_ps = tp.tile([128, 128], BF16, tag="tps")
            nc.tensor.transpose(t_ps[:], h_bf[:, j * 128:(j + 1) * 128],
                                id128[:])
            nc.vector.tensor_copy(hT[:, j, :], t_ps[:])

        # FC: raw.T = gw.T @ hT  (16, 64); row 10 = sum(h)
        f_ps = fp.tile([C2, BL], F32, tag="fps")
        for k in range(KC):
            nc.tensor.matmul(f_ps[:], gw_sb[:, k, :], hT_sl(hT, k),
                             start=(k == 0), stop=(k == KC - 1))
        fc_sb = sp.tile([C2, BL], F32, tag="fc_sb")
        nc.vector.tensor_copy(fc_sb[:], f_ps[:])
        l_ps = lp.tile([BL, C2], F32, tag="lps")
        nc.tensor.transpose(l_ps[:], fc_sb[:], id16[:])

        # fold sumsq: rows 0:64 + rows 64:128
        st_lo = sp.tile([BL, 1], F32, tag="st_lo")
        nc.sync.dma_start(st_lo[:], st2[BL:128, :])
        ssq = sp.tile([BL, 1], F32, tag="ssq")
        nc.vector.tensor_tensor(ssq[:], st2[0:BL, :], st_lo[:], OP.add)

        # mu = sum/H ; var = sumsq/H - mu^2 ; rsig = 1/sqrt(var+eps)
        mu = sp.tile([BL, 1], F32, tag="mu")
        nc.vector.tensor_scalar(mu[:], l_ps[:, C:C + 1], 1.0 / H, None,
                                OP.mult)
        musq = sp.tile([BL, 1], F32, tag="musq")
        nc.vector.tensor_tensor(musq[:], mu[:], mu[:], OP.mult)
        var = sp.tile([BL, 1], F32, tag="var")
        nc.vector.tensor_scalar(var[:], ssq[:], 1.0 / H, None, OP.mult)
        nc.vector.tensor_tensor(var[:], var[:], musq[:], OP.subtract)
        sd = sp.tile([BL, 1], F32, tag="sd")
        act(sd[:], var[:], AF.Sqrt, bias=eps_sb[:])
        rsig = sp.tile([BL, 1], F32, tag="rsig")
        nc.vector.reciprocal(rsig[:], sd[:])

        # logits = rsig*(raw - mu*u) + vbb
        t3 = sp.tile([BL, C], F32, tag="t3")
        nc.vector.tensor_scalar_mul(t3[:], ub_sb[:], mu[:])
        t4 = sp.tile([BL, C], F32, tag="t4")
        nc.vector.tensor_tensor(t4[:], l_ps[:, 0:C], t3[:], OP.subtract)
        nc.vector.tensor_scalar_mul(t4[:], t4[:], rsig[:])
        nc.vector.tensor_tensor(logit_acc[:, t * C:(t + 1) * C], t4[:],
                                vb_sb[:], OP.add)

        hT_prev = hT
        xts.pop(t, None)

    nc.sync.dma_start(dOUT[:], logit_acc[:])


def build(T=T_FULL, with_bias=False):
    from contextlib import ExitStack

    nc = bacc.Bacc("TRN2", target_bir_lowering=False)
    with tile.TileContext(nc) as tc:
        with ExitStack() as ctx:
            emit(ctx, nc, tc, T, with_bias)
    nc.compile()
    return nc


def kernel(x, W, U, b, ln_g, ln_b, fc_w, fc_b, _T=None, _trace=False):
    x = np.asarray(x, dtype=np.float32)
    W = np.asarray(W, dtype=np.float32)
    U = np.asarray(U, dtype=np.float32)
    b = np.asarray(b, dtype=np.float32)
    ln_g = np.asarray(ln_g, dtype=np.float32)
    ln_b = np.asarray(ln_b, dtype=np.float32)
    fc_w = np.asarray(fc_w, dtype=np.float32)
    fc_b = np.asarray(fc_b, dtype=np.float32)

    T = _T or x.shape[1]
    with_bias = bool(np.any(b))

    W_all = np.concatenate([W[g] for g in range(4)], axis=1)  # (H, 4H)
    U_all = np.concatenate([U[g] for g in range(4)], axis=1)
    Wn = np.ascontiguousarray(
        W_all.reshape(KC, 128, G4)).astype(ml_dtypes.bfloat16)
    Un = np.ascontiguousarray(
        U_all.reshape(KC, 128, G4)).astype(ml_dtypes.bfloat16)
    gw_full = np.zeros((H, C2), dtype=np.float32)
    gw_full[:, 0:C] = ln_g[:, None] * fc_w
    gw_full[:, C] = 1.0  # ones column -> sum(h)
    gw = gw_full.reshape(KC, 128, C2).astype(ml_dtypes.bfloat16)
    u_vec = (ln_g @ fc_w).astype(np.float32)  # (C,)
    vb = (ln_b @ fc_w + fc_b).astype(np.float32)
    ub_b = np.broadcast_to(u_vec, (BL, C)).copy()
    vb_b = np.broadcast_to(vb, (BL, C)).copy()

    common = {"Wn": Wn, "Un": Un, "gw": gw, "ub": ub_b, "vbb": vb_b}
    if with_bias:
        b_all = np.concatenate([b[g] for g in range(4)])  # (4H,)
        bb = np.zeros((128, G4), dtype=ml_dtypes.bfloat16)
        bb[0, :] = b_all.astype(ml_dtypes.bfloat16)
        common["bb"] = bb

    in_maps = []
    for ci in range(N_CORES):
        xc = x[ci * BL:(ci + 1) * BL, :T]           # (BL, T, H)
        xT = xc.transpose(1, 2, 0)                   # (T, H, BL)
        xT = np.ascontiguousarray(xT).reshape(T, KC, 128, BL)
        in_maps.append({"xT": xT.astype(ml_dtypes.bfloat16), **common})

    nc = build(T, with_bias)
    res = run_bass_kernel_spmd(nc, in_maps, core_ids=list(range(N_CORES)),
                               trace=_trace)
    out = np.concatenate(
        [r["out"].reshape(BL, T, C) for r in res.results], axis=0)
    if _trace:
        kernel.last_exec_time_ns = res.exec_time_ns
    return out
